# revision 23
# baseline (speedup 1.0000x reference)
"""Trainium2 Bass kernel for nn_PiProducts.

Reference computation (G=512 graphs, T=128 terms, P=256 params, B=512 batch):
    psi = (psi_const + einsum('gtp,bp->bgt', psi_params, w)) mod 2
    phi = (phi_const + einsum('gtp,bp->bgt', phi_params, w)) mod 2
    out[b,g,:] = (1 - 2*((sum_t psi*phi) mod 2)) * [1,0,0,0]

Algebraic reduction (validated exact): with u = A_g w, v = B_g w (raw integer
matmuls, no mod), E_g = (psi_const_g^T B_g + phi_const_g^T A_g) mod 2,
f_g = (psi_const_g . phi_const_g) mod 2:
    parity(sum_t psi*phi) == parity(sum_t u_t v_t + E_g.w_b + f_g)
All intermediates are small integers: u,v <= 104 (bf16-exact inputs),
s < 2^24 (fp32-exact), so bf16 tables + fp32 PSUM give zero numeric error.

Sharding: graphs split across 8 cores (64 graphs/core), param_vals replicated.

Engine split per 512-col tile (4 graphs):
  PATH A: ACT copies U PSUM->SBUF; DVE does 4 fused product+reduce
          (scalar_tensor_tensor with accum_out) reading V from PSUM.
  PATH C: ACT copies U and V PSUM->SBUF; Pool does the elementwise product;
          DVE does one grouped tensor_reduce (axis=X) over (128,4,128).
Parity: f32->int32 copy, bitwise_and 1, affine to +-1 (no mod ALU on TRN2).
"""

from contextlib import ExitStack

import ml_dtypes
import numpy as np

import concourse.bacc as bacc
import concourse.tile as tile
from concourse import mybir
from concourse.bass import ds, ts
from concourse.bass_utils import run_bass_kernel_spmd

G, T, P, B = 512, 128, 256, 512
NCORES = 8
GL = G // NCORES          # graphs per core
NT = GL * T               # gt columns per core (8192)
NTILES = NT // 512        # 16 moving tiles of 512 columns (4 graphs each)
MB = B // 128             # 4 batch chunks

# tile indices using PATH C (Pool product + DVE grouped reduce);
# the rest use PATH A (DVE fused product+reduce)
PATHC = frozenset({1, 3, 5, 7, 9, 11, 13, 15})

_BF16 = ml_dtypes.bfloat16
_FP8 = ml_dtypes.float8_e4m3
_NC = None


def _matmul_noload(nc, *args, **kw):
    """Emit a matmul whose InstMatmult carries ldweights=False at
    construction time (post-hoc mutation of .ins does not survive
    registration), so walrus skips the redundant LDWEIGHTS when the PE
    array already holds the same stationary weights."""
    orig = mybir.InstMatmult

    def patched(**k):
        k["ldweights"] = False
        return orig(**k)

    mybir.InstMatmult = patched
    try:
        return nc.tensor.matmul(*args, **kw)
    finally:
        mybir.InstMatmult = orig


def _build_nc():
    nc = bacc.Bacc("TRN2", target_bir_lowering=False, debug=False,
                   num_devices=NCORES)
    dt = mybir.dt
    f32, bf16, i32, fp8 = dt.float32, dt.bfloat16, dt.int32, dt.float8e4
    alu = mybir.AluOpType
    DR = mybir.MatmulPerfMode.DoubleRow

    # fp8 DoubleRow tables: layout [p, ntile, khalf, 512] flattened
    af = nc.dram_tensor("af", [128, 2 * NT], fp8, kind="ExternalInput").ap()
    bf = nc.dram_tensor("bf", [128, 2 * NT], fp8, kind="ExternalInput").ap()
    # fp8 DoubleRow stationary: layout [p, mchunk, khalf, 128] flattened
    wd = nc.dram_tensor("wd", [128, 2 * B], fp8, kind="ExternalInput").ap()
    ew = nc.dram_tensor("ew", [B, GL], f32, kind="ExternalInput").ap()
    out = nc.dram_tensor("out", [B, GL], f32, kind="ExternalOutput").ap()

    with tile.TileContext(nc) as tc, ExitStack() as ctx:
        tab = ctx.enter_context(tc.tile_pool(name="tab", bufs=1))
        psum_uv = ctx.enter_context(
            tc.tile_pool(name="psum_uv", bufs=8, space="PSUM"))
        work = ctx.enter_context(tc.tile_pool(name="work", bufs=14))

        afs = tab.tile([128, 2 * NT], fp8, tag="afs", name="afs")
        bfs = tab.tile([128, 2 * NT], fp8, tag="bfs", name="bfs")
        nc.gpsimd.dma_start(afs[:], af[:])
        nc.gpsimd.dma_start(bfs[:], bf[:])

        wds = tab.tile([128, 2 * B], fp8, tag="wds", name="wds")
        nc.sync.dma_start(wds[:], wd[:])
        ews = tab.tile([128, MB * GL], f32, tag="ews", name="ews")
        for m in range(MB):
            nc.sync.dma_start(ews[:, ds(m * GL, GL)], ew[ds(m * 128, 128), :])

        for m in range(MB):
            z = work.tile([128, GL], f32, tag="z", name=f"z{m}")
            lhs = wds[:, ds(m * 256, 256)].rearrange(
                "p (two m) -> p two m", two=2)
            for n in range(NTILES):
                ncol = ds(n * 1024, 1024)
                U = psum_uv.tile([128, 512], f32, tag="uv", name=f"u{m}_{n}")
                V = psum_uv.tile([128, 512], f32, tag="uv", name=f"v{m}_{n}")
                rhs_a = afs[:, ncol].rearrange("p (n two) -> p two n", two=2)
                rhs_b = bfs[:, ncol].rearrange("p (n two) -> p two n", two=2)
                nc.tensor.matmul(U[:], lhs, rhs_a,
                                 start=True, stop=True, perf_mode=DR)
                _matmul_noload(nc, V[:], lhs, rhs_b,
                               start=True, stop=True, perf_mode=DR)
                usb = work.tile([128, 512], f32, tag="usb", name=f"usb{m}_{n}")
                nc.scalar.copy(usb[:], U[:])
                if n in PATHC:
                    vsb = work.tile([128, 512], f32, tag="vsb",
                                    name=f"vsb{m}_{n}")
                    nc.scalar.copy(vsb[:], V[:])
                    prod = work.tile([128, 512], f32, tag="prod",
                                     name=f"prod{m}_{n}")
                    nc.gpsimd.tensor_tensor(out=prod[:], in0=usb[:],
                                            in1=vsb[:], op=alu.mult)
                    nc.vector.tensor_reduce(
                        out=z[:, ds(n * 4, 4)],
                        in_=prod[:].rearrange("p (g t) -> p g t", g=4),
                        axis=mybir.AxisListType.X, op=alu.add)
                else:
                    dump = work.tile([128, T], f32, tag="dump",
                                     name=f"dump{m}_{n}")
                    for j in range(4):
                        g = 4 * n + j
                        nc.vector.scalar_tensor_tensor(
                            out=dump[:], in0=usb[:, ds(j * T, T)],
                            scalar=1.0, in1=V[:, ds(j * T, T)],
                            op0=alu.mult, op1=alu.mult,
                            accum_out=z[:, ds(g, 1)])

            s = work.tile([128, GL], f32, tag="s", name=f"s{m}")
            nc.vector.tensor_tensor(out=s[:], in0=z[:],
                                    in1=ews[:, ds(m * GL, GL)], op=alu.add)
            si = work.tile([128, GL], i32, tag="si", name=f"si{m}")
            nc.vector.tensor_copy(si[:], s[:])
            sb = work.tile([128, GL], i32, tag="sb", name=f"sb{m}")
            nc.vector.tensor_scalar(out=sb[:], in0=si[:], scalar1=1,
                                    scalar2=None, op0=alu.bitwise_and)
            obuf = work.tile([128, GL], f32, tag="o", name=f"o{m}")
            nc.vector.tensor_scalar(out=obuf[:], in0=sb[:], scalar1=-2.0,
                                    scalar2=1.0, op0=alu.mult, op1=alu.add)
            nc.sync.dma_start(out[ds(m * 128, 128), :], obuf[:])

    nc.finalize()
    return nc


def _get_nc():
    global _NC
    if _NC is None:
        _NC = _build_nc()
    return _NC


def _prep_in_maps(inputs):
    psi_params = np.asarray(inputs["psi_params"], dtype=np.float32)
    phi_params = np.asarray(inputs["phi_params"], dtype=np.float32)
    psi_const = np.asarray(inputs["psi_const"], dtype=np.float32)
    phi_const = np.asarray(inputs["phi_const"], dtype=np.float32)
    param_vals = np.asarray(inputs["param_vals"], dtype=np.float32)

    E = (np.einsum("gt,gtp->gp", psi_const, phi_params, optimize=True)
         + np.einsum("gt,gtp->gp", phi_const, psi_params, optimize=True))
    E = np.mod(E, 2.0)
    f = np.mod((psi_const * phi_const).sum(axis=1), 2.0)
    wtf = np.ascontiguousarray(param_vals.T)                   # (P, B) f32
    # DoubleRow stationary: wd[p, m*256 + i*128 + j] = wt[i*128+p, m*128+j]
    wd = np.ascontiguousarray(
        wtf.reshape(2, 128, MB, 128).transpose(1, 2, 0, 3).reshape(128, 2 * B)
    ).astype(_FP8)
    # host E.w + f: (B, G) f32, exact (values <= 257)
    ewf = (param_vals.astype(np.float64) @ E.T.astype(np.float64)
           + f[None, :]).astype(np.float32)

    def dr_table(x):  # (P, NT) -> (128, 2*NT) pair-interleaved fp8
        return np.ascontiguousarray(
            x.reshape(2, 128, NTILES, 512).transpose(1, 2, 3, 0)
            .reshape(128, 2 * NT)).astype(_FP8)

    in_maps = []
    for c in range(NCORES):
        sl = slice(c * GL, (c + 1) * GL)
        at = np.transpose(psi_params[sl], (2, 0, 1)).reshape(P, NT)
        bt = np.transpose(phi_params[sl], (2, 0, 1)).reshape(P, NT)
        in_maps.append({
            "af": dr_table(at),
            "bf": dr_table(bt),
            "wd": wd,
            "ew": np.ascontiguousarray(ewf[:, sl]),
        })
    return in_maps


def run_device(inputs, trace=False):
    in_maps = _prep_in_maps(inputs)
    return run_bass_kernel_spmd(_get_nc(), in_maps, list(range(NCORES)),
                                trace=trace)


def kernel(**inputs) -> np.ndarray:
    res = run_device(inputs)
    sign = np.concatenate(
        [np.asarray(res.results[c]["out"]) for c in range(NCORES)], axis=1)
    full = np.zeros((B, G, 4), dtype=np.float32)
    full[:, :, 0] = sign.astype(np.float32)
    return full


# revision 28
# speedup vs baseline: 1.0118x; 1.0118x over previous
"""Trainium2 Bass kernel for nn_PiProducts.

Reference computation (G=512 graphs, T=128 terms, P=256 params, B=512 batch):
    psi = (psi_const + einsum('gtp,bp->bgt', psi_params, w)) mod 2
    phi = (phi_const + einsum('gtp,bp->bgt', phi_params, w)) mod 2
    out[b,g,:] = (1 - 2*((sum_t psi*phi) mod 2)) * [1,0,0,0]

Algebraic reduction (validated exact): with u = A_g w, v = B_g w (raw integer
matmuls, no mod), E_g = (psi_const_g^T B_g + phi_const_g^T A_g) mod 2,
f_g = (psi_const_g . phi_const_g) mod 2:
    parity(sum_t psi*phi) == parity(sum_t u_t v_t + E_g.w_b + f_g)
All intermediates are small integers: u,v <= 104 (bf16-exact inputs),
s < 2^24 (fp32-exact), so bf16 tables + fp32 PSUM give zero numeric error.

Sharding: graphs split across 8 cores (64 graphs/core), param_vals replicated.

Engine split per 512-col tile (4 graphs):
  PATH A: ACT copies U PSUM->SBUF; DVE does 4 fused product+reduce
          (scalar_tensor_tensor with accum_out) reading V from PSUM.
  PATH C: ACT copies U and V PSUM->SBUF; Pool does the elementwise product;
          DVE does one grouped tensor_reduce (axis=X) over (128,4,128).
Parity: f32->int32 copy, bitwise_and 1, affine to +-1 (no mod ALU on TRN2).
"""

from contextlib import ExitStack

import ml_dtypes
import numpy as np

import concourse.bacc as bacc
import concourse.tile as tile
from concourse import mybir
from concourse.bass import ds, ts
from concourse.bass_utils import run_bass_kernel_spmd

G, T, P, B = 512, 128, 256, 512
NCORES = 8
GL = G // NCORES          # graphs per core
NT = GL * T               # gt columns per core (8192)
NTILES = NT // 512        # 16 moving tiles of 512 columns (4 graphs each)
MB = B // 128             # 4 batch chunks

# tile indices using PATH C (Pool product + DVE grouped reduce);
# the rest use PATH A (DVE fused product+reduce)
PATHC = frozenset({1, 3, 5, 7, 9, 11, 13, 15})

_BF16 = ml_dtypes.bfloat16
_FP8 = ml_dtypes.float8_e4m3
_NC = None


def _matmul_noload(nc, *args, **kw):
    """Emit a matmul whose InstMatmult carries ldweights=False at
    construction time (post-hoc mutation of .ins does not survive
    registration), so walrus skips the redundant LDWEIGHTS when the PE
    array already holds the same stationary weights."""
    orig = mybir.InstMatmult

    def patched(**k):
        k["ldweights"] = False
        return orig(**k)

    mybir.InstMatmult = patched
    try:
        return nc.tensor.matmul(*args, **kw)
    finally:
        mybir.InstMatmult = orig


def _build_nc():
    nc = bacc.Bacc("TRN2", target_bir_lowering=False, debug=False,
                   num_devices=NCORES)
    dt = mybir.dt
    f32, bf16, i32, fp8 = dt.float32, dt.bfloat16, dt.int32, dt.float8e4
    alu = mybir.AluOpType
    DR = mybir.MatmulPerfMode.DoubleRow

    # fp8 DoubleRow tables: layout [p, ntile, khalf, 512] flattened
    af = nc.dram_tensor("af", [128, 2 * NT], fp8, kind="ExternalInput").ap()
    bf = nc.dram_tensor("bf", [128, 2 * NT], fp8, kind="ExternalInput").ap()
    # fp8 DoubleRow stationary: layout [p, mchunk, khalf, 128] flattened
    wd = nc.dram_tensor("wd", [128, 2 * B], fp8, kind="ExternalInput").ap()
    ew = nc.dram_tensor("ew", [B, GL], f32, kind="ExternalInput").ap()
    out = nc.dram_tensor("out", [B, GL], f32, kind="ExternalOutput").ap()

    with tile.TileContext(nc) as tc, ExitStack() as ctx:
        tab = ctx.enter_context(tc.tile_pool(name="tab", bufs=1))
        psum_uv = ctx.enter_context(
            tc.tile_pool(name="psum_uv", bufs=8, space="PSUM"))
        work = ctx.enter_context(tc.tile_pool(name="work", bufs=10))

        afs = tab.tile([128, 2 * NT], fp8, tag="afs", name="afs")
        bfs = tab.tile([128, 2 * NT], fp8, tag="bfs", name="bfs")
        nc.gpsimd.dma_start(afs[:], af[:])
        nc.gpsimd.dma_start(bfs[:], bf[:])

        wds = tab.tile([128, 2 * B], fp8, tag="wds", name="wds")
        nc.sync.dma_start(wds[:], wd[:])
        ews = tab.tile([128, MB * GL], f32, tag="ews", name="ews")
        for m in range(MB):
            nc.sync.dma_start(ews[:, ds(m * GL, GL)], ew[ds(m * 128, 128), :])

        for m in range(MB):
            z = work.tile([128, GL], f32, tag="z", name=f"z{m}")
            lhs = wds[:, ds(m * 256, 256)].rearrange(
                "p (two m) -> p two m", two=2)
            for n in range(NTILES):
                ncol = ds(n * 1024, 1024)
                U = psum_uv.tile([128, 512], f32, tag="uv", name=f"u{m}_{n}")
                V = psum_uv.tile([128, 512], f32, tag="uv", name=f"v{m}_{n}")
                rhs_a = afs[:, ncol].rearrange("p (n two) -> p two n", two=2)
                rhs_b = bfs[:, ncol].rearrange("p (n two) -> p two n", two=2)
                nc.tensor.matmul(U[:], lhs, rhs_a,
                                 start=True, stop=True, perf_mode=DR)
                _matmul_noload(nc, V[:], lhs, rhs_b,
                               start=True, stop=True, perf_mode=DR)
                usb = work.tile([128, 512], f32, tag="usb", name=f"usb{m}_{n}")
                nc.scalar.copy(usb[:], U[:])
                if n in PATHC:
                    vsb = work.tile([128, 512], f32, tag="vsb",
                                    name=f"vsb{m}_{n}")
                    nc.scalar.copy(vsb[:], V[:])
                    prod = work.tile([128, 512], f32, tag="prod",
                                     name=f"prod{m}_{n}")
                    nc.gpsimd.tensor_tensor(out=prod[:], in0=usb[:],
                                            in1=vsb[:], op=alu.mult)
                    nc.vector.tensor_reduce(
                        out=z[:, ds(n * 4, 4)],
                        in_=prod[:].rearrange("p (g t) -> p g t", g=4),
                        axis=mybir.AxisListType.X, op=alu.add)
                else:
                    dump = work.tile([128, T], f32, tag="dump",
                                     name=f"dump{m}_{n}")
                    for j in range(4):
                        g = 4 * n + j
                        nc.vector.scalar_tensor_tensor(
                            out=dump[:], in0=usb[:, ds(j * T, T)],
                            scalar=1.0, in1=V[:, ds(j * T, T)],
                            op0=alu.mult, op1=alu.mult,
                            accum_out=z[:, ds(g, 1)])

            s = work.tile([128, GL], f32, tag="s", name=f"s{m}")
            nc.vector.tensor_tensor(out=s[:], in0=z[:],
                                    in1=ews[:, ds(m * GL, GL)], op=alu.add)
            si = work.tile([128, GL], i32, tag="si", name=f"si{m}")
            nc.vector.tensor_copy(si[:], s[:])
            sb = work.tile([128, GL], i32, tag="sb", name=f"sb{m}")
            nc.vector.tensor_scalar(out=sb[:], in0=si[:], scalar1=1,
                                    scalar2=None, op0=alu.bitwise_and)
            obuf = work.tile([128, GL], f32, tag="o", name=f"o{m}")
            nc.vector.tensor_scalar(out=obuf[:], in0=sb[:], scalar1=-2.0,
                                    scalar2=1.0, op0=alu.mult, op1=alu.add)
            nc.sync.dma_start(out[ds(m * 128, 128), :], obuf[:])

    nc.finalize()
    return nc


def _get_nc():
    global _NC
    if _NC is None:
        _NC = _build_nc()
    return _NC


def _prep_in_maps(inputs):
    psi_params = np.asarray(inputs["psi_params"], dtype=np.float32)
    phi_params = np.asarray(inputs["phi_params"], dtype=np.float32)
    psi_const = np.asarray(inputs["psi_const"], dtype=np.float32)
    phi_const = np.asarray(inputs["phi_const"], dtype=np.float32)
    param_vals = np.asarray(inputs["param_vals"], dtype=np.float32)

    E = (np.einsum("gt,gtp->gp", psi_const, phi_params, optimize=True)
         + np.einsum("gt,gtp->gp", phi_const, psi_params, optimize=True))
    E = np.mod(E, 2.0)
    f = np.mod((psi_const * phi_const).sum(axis=1), 2.0)
    wtf = np.ascontiguousarray(param_vals.T)                   # (P, B) f32
    # DoubleRow stationary: wd[p, m*256 + i*128 + j] = wt[i*128+p, m*128+j]
    wd = np.ascontiguousarray(
        wtf.reshape(2, 128, MB, 128).transpose(1, 2, 0, 3).reshape(128, 2 * B)
    ).astype(_FP8)
    # host E.w + f: (B, G) f32, exact (values <= 257)
    ewf = (param_vals.astype(np.float64) @ E.T.astype(np.float64)
           + f[None, :]).astype(np.float32)

    def dr_table(x):  # (P, NT) -> (128, 2*NT) pair-interleaved fp8
        return np.ascontiguousarray(
            x.reshape(2, 128, NTILES, 512).transpose(1, 2, 3, 0)
            .reshape(128, 2 * NT)).astype(_FP8)

    in_maps = []
    for c in range(NCORES):
        sl = slice(c * GL, (c + 1) * GL)
        at = np.transpose(psi_params[sl], (2, 0, 1)).reshape(P, NT)
        bt = np.transpose(phi_params[sl], (2, 0, 1)).reshape(P, NT)
        in_maps.append({
            "af": dr_table(at),
            "bf": dr_table(bt),
            "wd": wd,
            "ew": np.ascontiguousarray(ewf[:, sl]),
        })
    return in_maps


def run_device(inputs, trace=False):
    in_maps = _prep_in_maps(inputs)
    return run_bass_kernel_spmd(_get_nc(), in_maps, list(range(NCORES)),
                                trace=trace)


def kernel(**inputs) -> np.ndarray:
    res = run_device(inputs)
    sign = np.concatenate(
        [np.asarray(res.results[c]["out"]) for c in range(NCORES)], axis=1)
    full = np.zeros((B, G, 4), dtype=np.float32)
    full[:, :, 0] = sign.astype(np.float32)
    return full
